# revision 30
# baseline (speedup 1.0000x reference)
"""Trainium2 Bass kernel for nn_CommNetActor (gnn_message_passing).

Algebraic collapse: every comm layer is linear (no activation), so the whole
post-sigmoid network folds into per-agent decoder matrices on the host:

    out[b] = sum_a sigmoid(O[b,a] @ W_enc + b_enc) @ Z_a + r

with Z_a = Gamma4 @ Wdec_a + E4 @ (sum_a' Wdec_a') and r = c4 @ Wsum + b_dec.

Device pipeline per core (batch-sharded, 8192/8 = 1024 batches = 65536 tokens,
O uploaded as bf16 [65536, 128]):
  - dma_start_transpose quad loads: [4096 tok, 128 f] DRAM -> [128 f, 4096] SBUF
    (XBAR 16x128 tiles) - no PE transpose, no downcast, half the HBM bytes
  - encoder: per 1024-token supertile, 2 col-tiled bf16 matmuls with stride-2
    moving APs split tokens by agent parity -> PSUM [128=(par,d), 512=(b,i)]
  - ACT sigmoid(psum + b_enc) -> bf16 arena [128, 512] per supertile
  - decoder: K=128 agent-pair matmuls (stationary [Zdec_2i; Zdec_2i+1]),
    32 matmuls x 256 batch-cols per 16-supertile group, PSUM accumulate
  - +r (ACT), PE transpose to token-major, DMA out
"""

import sys
import numpy as np

sys.path.insert(0, "/opt/trn_rl_repo")

import ml_dtypes

BATCH, N_AGENTS, OBS_DIM, D, N_ACT = 8192, 64, 128, 64, 32
N_CORES = 8
B_CORE = BATCH // N_CORES              # 1024 batches per core
TOK_CORE = B_CORE * N_AGENTS           # 65536 tokens per core
NT = 1024                              # tokens per super-tile (16 batches)
N_ST = TOK_CORE // NT                  # 64 super-tiles
QS = 4                                 # super-tiles per DMA quad
N_Q = N_ST // QS                       # 16 quads
SG = 16                                # super-tiles per decoder group
N_G = N_ST // SG                       # 4 groups
GB = SG * (NT // N_AGENTS)             # 256 batches per group
NPAIR = N_AGENTS // 2                  # 32 agent pairs

_CACHE = {}


def _fold_weights(W_enc, b_enc, Ws, bs, W_dec, b_dec):
    """Fold the 4 linear comm layers + decoder into Zdec [64,64,32] and r[32]."""
    A = N_AGENTS
    I = np.eye(D)
    Gamma = I.copy()
    E = np.zeros((D, D))
    c = np.zeros(D)
    Lam = I.copy()
    d = np.zeros(D)
    for W, b in zip(Ws, bs):
        W = W.astype(np.float64)
        b = b.astype(np.float64)
        Wt, Wb = W[:D], W[D:]
        V = Wb / A
        Wp = Wt - V
        U = Wt + (A - 1) * V
        E, c = E @ Wp + Lam @ V, c @ Wp + d @ V + b
        Gamma = Gamma @ Wp
        Lam, d = Lam @ U, d @ U + A * b
    Wd = W_dec.astype(np.float64).reshape(A, D, N_ACT)
    Wsum = Wd.sum(axis=0)
    Zdec = np.einsum("ij,ajk->aik", Gamma, Wd) + (E @ Wsum)[None]
    r = c @ Wsum + b_dec.astype(np.float64)
    return Zdec, r


def _build(loop_reps=1, ot_bufs=3, ph_bufs=2, store_eng="sync",
           dec_q=2, ar_bufs=2, unroll=1, wide_ph=True, hi_pri_loads=True,
           pri_off=None, store_late=False):
    import concourse.bass as bass
    import concourse.bacc as bacc
    import concourse.tile as tile
    from concourse import mybir
    from concourse._compat import get_trn_type

    f32 = mybir.dt.float32
    bf16 = mybir.dt.bfloat16

    nc = bacc.Bacc(get_trn_type() or "TRN2", target_bir_lowering=False,
                   debug=False, enable_asserts=True, num_devices=N_CORES)

    O_d = nc.dram_tensor("O", [TOK_CORE, OBS_DIM], bf16, kind="ExternalInput")
    wenc_d = nc.dram_tensor("Wenc", [OBS_DIM, D], bf16, kind="ExternalInput")
    benc_d = nc.dram_tensor("benc2", [128, 1], f32, kind="ExternalInput")
    zdec_d = nc.dram_tensor("Zdec2", [128, NPAIR, N_ACT], bf16,
                            kind="ExternalInput")
    r_d = nc.dram_tensor("r2", [N_ACT, 1], f32, kind="ExternalInput")
    idf_d = nc.dram_tensor("identf", [N_ACT, N_ACT], f32, kind="ExternalInput")
    out_d = nc.dram_tensor("out", [B_CORE, N_ACT], f32, kind="ExternalOutput")

    O_ap = O_d.ap()
    out_ap = out_d.ap()

    with tile.TileContext(nc) as tc:
        with (
            tc.tile_pool(name="const", bufs=1) as const_pool,
            tc.tile_pool(name="ot", bufs=ot_bufs) as ot_pool,
            tc.tile_pool(name="arena", bufs=ar_bufs) as arena_pool,
            tc.tile_pool(name="outsb", bufs=2) as outsb_pool,
            tc.tile_pool(name="ph", bufs=ph_bufs, space="PSUM") as ph_pool,
            tc.tile_pool(name="pd", bufs=2, space="PSUM") as pd_pool,
            tc.tile_pool(name="po", bufs=1, space="PSUM") as po_pool,
        ):
            # constants
            wenc = const_pool.tile([OBS_DIM, D], bf16)
            nc.sync.dma_start(out=wenc[:], in_=wenc_d.ap())
            benc = const_pool.tile([128, 1], f32)
            nc.sync.dma_start(out=benc[:], in_=benc_d.ap())
            zdec = const_pool.tile([128, NPAIR, N_ACT], bf16)
            nc.sync.dma_start(out=zdec[:], in_=zdec_d.ap())
            r2 = const_pool.tile([N_ACT, 1], f32)
            nc.sync.dma_start(out=r2[:], in_=r_d.ap())
            identf = const_pool.tile([N_ACT, N_ACT], f32)
            nc.sync.dma_start(out=identf[:], in_=idf_d.ap())

            import contextlib
            loop_cm = (tc.For_i(0, loop_reps, 1) if loop_reps > 1
                       else contextlib.nullcontext())
            pending_store = []

            def flush_stores():
                store = getattr(nc, store_eng)
                old_pri = tc.cur_priority
                if store_late:
                    tc.cur_priority = old_pri + 100000
                for outt_t, gs in pending_store:
                    for ch in range(2):
                        store.dma_start(
                            out=out_ap[(gs * 2 + ch) * 128:
                                       (gs * 2 + ch + 1) * 128, :],
                            in_=outt_t[:, ch * N_ACT:(ch + 1) * N_ACT])
                if store_late:
                    tc.cur_priority = old_pri
                pending_store.clear()

            with loop_cm:
             for g in range(N_G * unroll):
                g = g % N_G
                pd = pd_pool.tile([N_ACT, GB], f32)
                for stx in range((SG // QS) // dec_q):
                    arena = arena_pool.tile([128, dec_q * QS * 512], bf16)
                    for ql in range(dec_q):
                        q = g * (SG // QS) + stx * dec_q + ql
                        # ---- XBAR dma transpose: [4096 tok, 128 f] -> [128, 4096]
                        ot4 = ot_pool.tile([128, QS * NT], bf16)
                        import contextlib as _cl
                        pri = (tc.high_priority(pri_off) if hi_pri_loads
                               else _cl.nullcontext())
                        with pri:
                            nc.sync.dma_start_transpose(
                                ot4[:], O_ap[q * QS * NT:(q + 1) * QS * NT, :])
                        if ql == 0 and stx == 0:
                            flush_stores()  # prior group's stores after 1 load
                        # moving view: [p, s, par, b, i]
                        otv = ot4[:].rearrange(
                            "p (s b i par) -> p s par b i",
                            s=QS, b=NT // N_AGENTS, par=2)
                        if wide_ph:
                            for sh in range(QS // 2):
                                eph = ph_pool.tile([128, 1024], f32)
                                for sx in range(2):
                                    s = sh * 2 + sx
                                    for par in range(2):
                                        nc.tensor.matmul(
                                            eph[par * D:(par + 1) * D,
                                                sx * 512:(sx + 1) * 512],
                                            wenc[:], otv[:, s, par, :, :],
                                            start=True, stop=True,
                                            tile_position=(0, par * D))
                                sl = ql * QS + sh * 2
                                nc.scalar.activation(
                                    out=arena[:, sl * 512:(sl + 2) * 512],
                                    in_=eph[:],
                                    func=mybir.ActivationFunctionType.Sigmoid,
                                    bias=benc[:],
                                )
                        else:
                            for s in range(QS):
                                # ---- encoder: 2 col-tiled matmuls, parity split
                                eph = ph_pool.tile([128, 512], f32)
                                for par in range(2):
                                    nc.tensor.matmul(
                                        eph[par * D:(par + 1) * D, :],
                                        wenc[:], otv[:, s, par, :, :],
                                        start=True, stop=True,
                                        tile_position=(0, par * D))
                                # ---- sigmoid(x + b_enc) -> bf16 arena
                                sl = ql * QS + s
                                nc.scalar.activation(
                                    out=arena[:, sl * 512:(sl + 1) * 512],
                                    in_=eph[:],
                                    func=mybir.ActivationFunctionType.Sigmoid,
                                    bias=benc[:],
                                )

                    # ---- decoder strip: K=128 agent-pair matmuls
                    av = arena[:].rearrange(
                        "p (st b i) -> p st b i", b=NT // N_AGENTS, i=NPAIR)
                    col0 = stx * dec_q * QS * 16
                    ncol = dec_q * QS * 16
                    for i in range(NPAIR):
                        nc.tensor.matmul(
                            pd[:, col0:col0 + ncol], zdec[:, i, :],
                            av[:, :, :, i],
                            start=(i == 0), stop=(i == NPAIR - 1))

                # ---- + r, transpose to token-major, store
                sab = outsb_pool.tile([N_ACT, GB], f32, tag="sab")
                nc.scalar.add(sab[:], pd[:], add=r2[:])
                po = po_pool.tile([128, 64], f32)
                for ch in range(2):
                    nc.tensor.transpose(
                        po[:, ch * N_ACT:(ch + 1) * N_ACT],
                        sab[:, ch * 128:(ch + 1) * 128],
                        identf[:])
                outt = outsb_pool.tile([128, 64], f32, tag="outt")
                nc.vector.tensor_copy(outt[:], po[:])
                pending_store.append((outt, g))
             flush_stores()

    nc.compile()
    return nc


def _prep_inputs(inputs):
    W_enc = np.asarray(inputs["W_enc"], dtype=np.float32)
    b_enc = np.asarray(inputs["b_enc"], dtype=np.float32)
    Ws = [np.asarray(inputs[f"W{k}"], dtype=np.float32) for k in (1, 2, 3, 4)]
    bs = [np.asarray(inputs[f"b{k}"], dtype=np.float32) for k in (1, 2, 3, 4)]
    W_dec = np.asarray(inputs["W_dec"], dtype=np.float32)
    b_dec = np.asarray(inputs["b_dec"], dtype=np.float32)

    Zdec, r = _fold_weights(W_enc, b_enc, Ws, bs, W_dec, b_dec)
    # agent-pair stationary: [Zdec_{2i}; Zdec_{2i+1}] stacked on partitions
    zdec2 = np.empty((128, NPAIR, N_ACT), dtype=np.float64)
    for i in range(NPAIR):
        zdec2[:D, i] = Zdec[2 * i]
        zdec2[D:, i] = Zdec[2 * i + 1]
    zdec2 = zdec2.astype(ml_dtypes.bfloat16)
    benc2 = np.concatenate([b_enc, b_enc]).reshape(128, 1).astype(np.float32)
    r2 = r.reshape(N_ACT, 1).astype(np.float32)

    O = np.asarray(inputs["O"], dtype=np.float32)
    common = {
        "Wenc": np.ascontiguousarray(W_enc).astype(ml_dtypes.bfloat16),
        "benc2": benc2,
        "Zdec2": zdec2,
        "r2": r2,
        "identf": np.eye(N_ACT, dtype=np.float32),
    }
    in_maps = []
    for c in range(N_CORES):
        o_shard = np.ascontiguousarray(
            O[c * B_CORE:(c + 1) * B_CORE].reshape(TOK_CORE, OBS_DIM)
        ).astype(ml_dtypes.bfloat16)
        in_maps.append({"O": o_shard, **common})
    return in_maps


def _run(inputs, trace=False):
    from concourse.bass_utils import run_bass_kernel_spmd

    if "nc" not in _CACHE:
        _CACHE["nc"] = _build()
    nc = _CACHE["nc"]
    in_maps = _prep_inputs(inputs)
    res = run_bass_kernel_spmd(nc, in_maps, core_ids=list(range(N_CORES)),
                               trace=trace)
    out = np.concatenate(
        [res.results[c]["out"] for c in range(N_CORES)], axis=0)
    return out.astype(np.float32), res


def kernel(**inputs):
    out, _ = _run(inputs, trace=False)
    return out


# revision 33
# speedup vs baseline: 20.9441x; 20.9441x over previous
"""Trainium2 Bass kernel for nn_CommNetActor (gnn_message_passing).

Algebraic collapse: every comm layer is linear (no activation), so the whole
post-sigmoid network folds into per-agent decoder matrices on the host:

    out[b] = sum_a sigmoid(O[b,a] @ W_enc + b_enc) @ Z_a + r

with Z_a = Gamma4 @ Wdec_a + E4 @ (sum_a' Wdec_a') and r = c4 @ Wsum + b_dec.

Device pipeline per core (batch-sharded, 8192/8 = 1024 batches = 65536 tokens,
O uploaded as bf16 [65536, 128]):
  - dma_start_transpose quad loads: [4096 tok, 128 f] DRAM -> [128 f, 4096] SBUF
    (XBAR 16x128 tiles) - no PE transpose, no downcast, half the HBM bytes
  - encoder: per 1024-token supertile, 2 col-tiled bf16 matmuls with stride-2
    moving APs split tokens by agent parity -> PSUM [128=(par,d), 512=(b,i)]
  - ACT sigmoid(psum + b_enc) -> bf16 arena [128, 512] per supertile
  - decoder: K=128 agent-pair matmuls (stationary [Zdec_2i; Zdec_2i+1]),
    32 matmuls x 256 batch-cols per 16-supertile group, PSUM accumulate
  - +r (ACT), PE transpose to token-major, DMA out
"""

import sys
import numpy as np

sys.path.insert(0, "/opt/trn_rl_repo")

import ml_dtypes

BATCH, N_AGENTS, OBS_DIM, D, N_ACT = 8192, 64, 128, 64, 32
N_CORES = 8
B_CORE = BATCH // N_CORES              # 1024 batches per core
TOK_CORE = B_CORE * N_AGENTS           # 65536 tokens per core
NT = 1024                              # tokens per super-tile (16 batches)
N_ST = TOK_CORE // NT                  # 64 super-tiles
QS = 4                                 # super-tiles per DMA quad
N_Q = N_ST // QS                       # 16 quads
SG = 16                                # super-tiles per decoder group
N_G = N_ST // SG                       # 4 groups
GB = SG * (NT // N_AGENTS)             # 256 batches per group
NPAIR = N_AGENTS // 2                  # 32 agent pairs

_CACHE = {}


def _fold_weights(W_enc, b_enc, Ws, bs, W_dec, b_dec):
    """Fold the 4 linear comm layers + decoder into Zdec [64,64,32] and r[32]."""
    A = N_AGENTS
    I = np.eye(D)
    Gamma = I.copy()
    E = np.zeros((D, D))
    c = np.zeros(D)
    Lam = I.copy()
    d = np.zeros(D)
    for W, b in zip(Ws, bs):
        W = W.astype(np.float64)
        b = b.astype(np.float64)
        Wt, Wb = W[:D], W[D:]
        V = Wb / A
        Wp = Wt - V
        U = Wt + (A - 1) * V
        E, c = E @ Wp + Lam @ V, c @ Wp + d @ V + b
        Gamma = Gamma @ Wp
        Lam, d = Lam @ U, d @ U + A * b
    Wd = W_dec.astype(np.float64).reshape(A, D, N_ACT)
    Wsum = Wd.sum(axis=0)
    Zdec = np.einsum("ij,ajk->aik", Gamma, Wd) + (E @ Wsum)[None]
    r = c @ Wsum + b_dec.astype(np.float64)
    return Zdec, r


def _build(loop_reps=1, ot_bufs=3, ph_bufs=2, store_eng="sync",
           dec_q=2, ar_bufs=2, unroll=1, wide_ph=True, hi_pri_loads=True,
           pri_off=None, store_late=False, qs=QS):
    import concourse.bass as bass
    import concourse.bacc as bacc
    import concourse.tile as tile
    from concourse import mybir
    from concourse._compat import get_trn_type

    f32 = mybir.dt.float32
    bf16 = mybir.dt.bfloat16

    nc = bacc.Bacc(get_trn_type() or "TRN2", target_bir_lowering=False,
                   debug=False, enable_asserts=True, num_devices=N_CORES)

    O_d = nc.dram_tensor("O", [TOK_CORE, OBS_DIM], bf16, kind="ExternalInput")
    wenc_d = nc.dram_tensor("Wenc", [OBS_DIM, D], bf16, kind="ExternalInput")
    benc_d = nc.dram_tensor("benc2", [128, 1], f32, kind="ExternalInput")
    zdec_d = nc.dram_tensor("Zdec2", [128, NPAIR, N_ACT], bf16,
                            kind="ExternalInput")
    r_d = nc.dram_tensor("r2", [N_ACT, 1], f32, kind="ExternalInput")
    idf_d = nc.dram_tensor("identf", [N_ACT, N_ACT], f32, kind="ExternalInput")
    out_d = nc.dram_tensor("out", [B_CORE, N_ACT], f32, kind="ExternalOutput")

    O_ap = O_d.ap()
    out_ap = out_d.ap()

    with tile.TileContext(nc) as tc:
        with (
            tc.tile_pool(name="const", bufs=1) as const_pool,
            tc.tile_pool(name="ot", bufs=ot_bufs) as ot_pool,
            tc.tile_pool(name="arena", bufs=ar_bufs) as arena_pool,
            tc.tile_pool(name="outsb", bufs=2) as outsb_pool,
            tc.tile_pool(name="ph", bufs=ph_bufs, space="PSUM") as ph_pool,
            tc.tile_pool(name="pd", bufs=2, space="PSUM") as pd_pool,
            tc.tile_pool(name="po", bufs=1, space="PSUM") as po_pool,
        ):
            # constants
            wenc = const_pool.tile([OBS_DIM, D], bf16)
            nc.sync.dma_start(out=wenc[:], in_=wenc_d.ap())
            benc = const_pool.tile([128, 1], f32)
            nc.sync.dma_start(out=benc[:], in_=benc_d.ap())
            zdec = const_pool.tile([128, NPAIR, N_ACT], bf16)
            nc.sync.dma_start(out=zdec[:], in_=zdec_d.ap())
            r2 = const_pool.tile([N_ACT, 1], f32)
            nc.sync.dma_start(out=r2[:], in_=r_d.ap())
            identf = const_pool.tile([N_ACT, N_ACT], f32)
            nc.sync.dma_start(out=identf[:], in_=idf_d.ap())

            import contextlib
            loop_cm = (tc.For_i(0, loop_reps, 1) if loop_reps > 1
                       else contextlib.nullcontext())
            pending_store = []

            def flush_stores():
                store = getattr(nc, store_eng)
                old_pri = tc.cur_priority
                if store_late:
                    tc.cur_priority = old_pri + 100000
                for outt_t, gs in pending_store:
                    for ch in range(2):
                        store.dma_start(
                            out=out_ap[(gs * 2 + ch) * 128:
                                       (gs * 2 + ch + 1) * 128, :],
                            in_=outt_t[:, ch * N_ACT:(ch + 1) * N_ACT])
                if store_late:
                    tc.cur_priority = old_pri
                pending_store.clear()

            with loop_cm:
             for g in range(N_G * unroll):
                g = g % N_G
                pd = pd_pool.tile([N_ACT, GB], f32)
                for stx in range((SG // qs) // dec_q):
                    arena = arena_pool.tile([128, dec_q * qs * 512], bf16)
                    for ql in range(dec_q):
                        q = g * (SG // qs) + stx * dec_q + ql
                        # ---- XBAR dma transpose: [4096 tok, 128 f] -> [128, 4096]
                        ot4 = ot_pool.tile([128, qs * NT], bf16)
                        import contextlib as _cl
                        pri = (tc.high_priority(pri_off) if hi_pri_loads
                               else _cl.nullcontext())
                        with pri:
                            nc.sync.dma_start_transpose(
                                ot4[:], O_ap[q * qs * NT:(q + 1) * qs * NT, :])
                        if ql == 0 and stx == 0:
                            flush_stores()  # prior group's stores after 1 load
                        # moving view: [p, s, par, b, i]
                        otv = ot4[:].rearrange(
                            "p (s b i par) -> p s par b i",
                            s=qs, b=NT // N_AGENTS, par=2)
                        if wide_ph:
                            for sh in range(qs // 2):
                                eph = ph_pool.tile([128, 1024], f32)
                                for sx in range(2):
                                    s = sh * 2 + sx
                                    for par in range(2):
                                        nc.tensor.matmul(
                                            eph[par * D:(par + 1) * D,
                                                sx * 512:(sx + 1) * 512],
                                            wenc[:], otv[:, s, par, :, :],
                                            start=True, stop=True,
                                            tile_position=(0, par * D))
                                sl = ql * qs + sh * 2
                                nc.scalar.activation(
                                    out=arena[:, sl * 512:(sl + 2) * 512],
                                    in_=eph[:],
                                    func=mybir.ActivationFunctionType.Sigmoid,
                                    bias=benc[:],
                                )
                        else:
                            for s in range(qs):
                                # ---- encoder: 2 col-tiled matmuls, parity split
                                eph = ph_pool.tile([128, 512], f32)
                                for par in range(2):
                                    nc.tensor.matmul(
                                        eph[par * D:(par + 1) * D, :],
                                        wenc[:], otv[:, s, par, :, :],
                                        start=True, stop=True,
                                        tile_position=(0, par * D))
                                # ---- sigmoid(x + b_enc) -> bf16 arena
                                sl = ql * qs + s
                                nc.scalar.activation(
                                    out=arena[:, sl * 512:(sl + 1) * 512],
                                    in_=eph[:],
                                    func=mybir.ActivationFunctionType.Sigmoid,
                                    bias=benc[:],
                                )

                    # ---- decoder strip: K=128 agent-pair matmuls
                    av = arena[:].rearrange(
                        "p (st b i) -> p st b i", b=NT // N_AGENTS, i=NPAIR)
                    col0 = stx * dec_q * qs * 16
                    ncol = dec_q * qs * 16
                    for i in range(NPAIR):
                        nc.tensor.matmul(
                            pd[:, col0:col0 + ncol], zdec[:, i, :],
                            av[:, :, :, i],
                            start=(i == 0), stop=(i == NPAIR - 1))

                # ---- + r, transpose to token-major, store
                sab = outsb_pool.tile([N_ACT, GB], f32, tag="sab")
                nc.scalar.add(sab[:], pd[:], add=r2[:])
                po = po_pool.tile([128, 64], f32)
                for ch in range(2):
                    nc.tensor.transpose(
                        po[:, ch * N_ACT:(ch + 1) * N_ACT],
                        sab[:, ch * 128:(ch + 1) * 128],
                        identf[:])
                outt = outsb_pool.tile([128, 64], f32, tag="outt")
                nc.vector.tensor_copy(outt[:], po[:])
                pending_store.append((outt, g))
             flush_stores()

    nc.compile()
    return nc


def _prep_inputs(inputs):
    W_enc = np.asarray(inputs["W_enc"], dtype=np.float32)
    b_enc = np.asarray(inputs["b_enc"], dtype=np.float32)
    Ws = [np.asarray(inputs[f"W{k}"], dtype=np.float32) for k in (1, 2, 3, 4)]
    bs = [np.asarray(inputs[f"b{k}"], dtype=np.float32) for k in (1, 2, 3, 4)]
    W_dec = np.asarray(inputs["W_dec"], dtype=np.float32)
    b_dec = np.asarray(inputs["b_dec"], dtype=np.float32)

    Zdec, r = _fold_weights(W_enc, b_enc, Ws, bs, W_dec, b_dec)
    # agent-pair stationary: [Zdec_{2i}; Zdec_{2i+1}] stacked on partitions
    zdec2 = np.empty((128, NPAIR, N_ACT), dtype=np.float64)
    for i in range(NPAIR):
        zdec2[:D, i] = Zdec[2 * i]
        zdec2[D:, i] = Zdec[2 * i + 1]
    zdec2 = zdec2.astype(ml_dtypes.bfloat16)
    benc2 = np.concatenate([b_enc, b_enc]).reshape(128, 1).astype(np.float32)
    r2 = r.reshape(N_ACT, 1).astype(np.float32)

    O = np.asarray(inputs["O"], dtype=np.float32)
    common = {
        "Wenc": np.ascontiguousarray(W_enc).astype(ml_dtypes.bfloat16),
        "benc2": benc2,
        "Zdec2": zdec2,
        "r2": r2,
        "identf": np.eye(N_ACT, dtype=np.float32),
    }
    in_maps = []
    for c in range(N_CORES):
        o_shard = np.ascontiguousarray(
            O[c * B_CORE:(c + 1) * B_CORE].reshape(TOK_CORE, OBS_DIM)
        ).astype(ml_dtypes.bfloat16)
        in_maps.append({"O": o_shard, **common})
    return in_maps


def _run(inputs, trace=False):
    from concourse.bass_utils import run_bass_kernel_spmd

    if "nc" not in _CACHE:
        _CACHE["nc"] = _build()
    nc = _CACHE["nc"]
    in_maps = _prep_inputs(inputs)
    res = run_bass_kernel_spmd(nc, in_maps, core_ids=list(range(N_CORES)),
                               trace=trace)
    out = np.concatenate(
        [res.results[c]["out"] for c in range(N_CORES)], axis=0)
    return out.astype(np.float32), res


def kernel(**inputs):
    out, _ = _run(inputs, trace=False)
    return out
